# revision 24
# baseline (speedup 1.0000x reference)
"""Barrier_Net TRN2 kernel: 8-core data-parallel Bass/Tile implementation (v2).

Per-core structure (12800 padded agents = 25 groups x 512):
  - Layer-1 MLPs: 12 fp16 matmuls/group (weights pre-scaled x32), relu evac
    to fp8e4 (values 32*relu) split across ScalarE/VectorE.
  - Layer-2 + rho1 fused by linearity (lhsT = (w2 @ rho_w1) * 8 in fp8),
    6 DoubleRow fp8 matmuls/group (contract 256) accumulate rho hidden
    pre-act (x256) in one PSUM tile.
  - rho2 + psi1 fused (lhsT rows 0:64 = rho_w2 @ psi_w1[:2] / 256), g rows
    appended to the same rhs tile -> single matmul.
  - empty-head e stashed per group into a [32,512] accumulator (16 groups),
    transposed once per chunk, tanh'd in batch (one act-table switch total).
  - barrier computed fully in fp32 (accuracy-critical) in a [128,1600]x2
    layout, neighbor reduce on DVE + partition fold via matmul with ones.
"""
import sys, os
sys.path.insert(0, "/opt/trn_rl_repo")
import numpy as np
import ml_dtypes
import concourse.bacc as bacc
import concourse.tile as tile
import concourse.mybir as mybir
from concourse.bass_utils import run_bass_kernel_spmd
from contextlib import ExitStack

F32 = mybir.dt.float32
F16 = mybir.dt.float16
F8 = mybir.dt.float8e4
AF = mybir.ActivationFunctionType
ALU = mybir.AluOpType
PM = mybir.MatmulPerfMode

B, NN, NO, SD = 100000, 16, 8, 4
H, PHI_OUT, ADIM = 64, 16, 2
DS, B_GAMMA = 0.2, 0.01
D_OBS = 85
NCORE = 8
AC = B // NCORE            # 12500 agents per core
G512 = 25                  # groups of 512
AP_ = G512 * 512           # padded agents per core = 12800
S1 = 32.0                  # layer-1 weight prescale
S2 = 8.0                   # fused layer-2 weight prescale
BARC = G512 * 16 * 4       # 1600 barrier cols per half

SCALAR_EVACS = frozenset({0, 1, 3, 4, 6, 7, 9, 10})


def _pack_weights(phi_w1, phi_b1, phi_w2, phi_b2, obs_w1, obs_b1, obs_w2, obs_b2,
                  rho_w1, rho_b1, rho_w2, rho_b2, psi_w1, psi_b1, psi_w2, psi_b2):
    # layer-1 lhsT: 12 K=32 row-tiles (strip = idx%4), prescaled x32
    w1all = np.zeros((128, 12 * 128), np.float32)
    for idx in range(8):
        sp, wv = idx % 4, idx // 4
        for f in range(8):
            d = f % 4
            w1all[32 * sp + 8 * wv + f,
                  idx * 128 + 64 * (f // 4):idx * 128 + 64 * (f // 4) + 64] = \
                phi_w1[d] * S1
    for q in range(4):
        idx = 8 + q
        for t in range(4):
            d = t % 2
            w1all[32 * q + 16 + t,
                  idx * 128 + 64 * (t // 2):idx * 128 + 64 * (t // 2) + 64] = \
                obs_w1[d] * S1
    # fused layer-2 + rho1 (DoubleRow lhsT [128, ko=2, 64]), x8
    W2R = np.tile((phi_w2 @ rho_w1) * S2, (2, 1))       # [128, 64]
    OW2R = np.tile((obs_w2 @ rho_w1) * S2, (2, 1))
    l2w = np.zeros((128, 2, 2, 64), np.float32)          # [p, which, ko, j]
    l2w[:, 0, 0] = l2w[:, 0, 1] = W2R
    l2w[:, 1, 0] = l2w[:, 1, 1] = OW2R
    # fused rho2+psi1 lhsT [66, 64]; rh carries a 1/(S1*S2) scale already
    PW = np.zeros((66, 64), np.float32)
    PW[0:64] = rho_w2 @ psi_w1[0:2]
    PW[64:66] = psi_w1[2:4]
    # bias columns
    RB1C = (rho_b1 + (NN * phi_b2 + NO * obs_b2) @ rho_w1).reshape(64, 1)
    PB1C = (psi_b1 + rho_b2 @ psi_w1[0:2]).reshape(64, 1)
    PB2C = psi_b2.reshape(2, 1)
    small = np.zeros((128, 8), np.float32)
    small[:, 0:1] = np.tile(phi_b1, 2).reshape(128, 1) * S1
    small[:, 1:2] = np.tile(obs_b1, 2).reshape(128, 1) * S1
    small[0:64, 2:3] = RB1C
    small[0:64, 3:4] = PB1C
    small[0:2, 4:5] = PB2C
    ones4 = np.zeros((128, 32), np.float32)
    for p in range(128):
        ones4[p, p % 32] = 1.0
    f8 = ml_dtypes.float8_e4m3
    PW2B = np.concatenate([psi_w2, psi_b2.reshape(1, 2)], axis=0)
    return {
        "w1all": w1all.astype(np.float16),
        "l2w": l2w.reshape(128, 256).astype(f8),
        "pw": PW.astype(np.float16),
        "pw2": PW2B.astype(np.float16),
        "ones4": ones4.astype(np.float16),
        "small": small.astype(np.float32),
    }


def _build(nc):
    xt_d = nc.dram_tensor("xt", [128, AP_], F16, kind="ExternalInput").ap()
    gg_d = nc.dram_tensor("gg", [2, AP_], F16, kind="ExternalInput").ap()
    xb_d = nc.dram_tensor("xb", [128, 2 * BARC], F32, kind="ExternalInput").ap()
    w1all_d = nc.dram_tensor("w1all", [128, 12 * 128], F16,
                             kind="ExternalInput").ap()
    l2w_d = nc.dram_tensor("l2w", [128, 256], F8, kind="ExternalInput").ap()
    pw_d = nc.dram_tensor("pw", [66, 64], F16, kind="ExternalInput").ap()
    pw2_d = nc.dram_tensor("pw2", [65, 2], F16, kind="ExternalInput").ap()
    ones4_d = nc.dram_tensor("ones4", [128, 32], F16, kind="ExternalInput").ap()
    small_d = nc.dram_tensor("small", [128, 8], F32, kind="ExternalInput").ap()
    y_d = nc.dram_tensor("y", [32, 1024], F32, kind="ExternalOutput").ap()

    with tile.TileContext(nc) as tc, ExitStack() as ctx, \
            nc.allow_low_precision(reason="fp16 barrier partials validated"):
        cw = ctx.enter_context(tc.tile_pool(name="cw", bufs=1))
        xin = ctx.enter_context(tc.tile_pool(name="xin", bufs=3))
        s8p = ctx.enter_context(tc.tile_pool(name="s8p", bufs=2))
        rgp = ctx.enter_context(tc.tile_pool(name="rgp", bufs=2))
        pep = ctx.enter_context(tc.tile_pool(name="pep", bufs=2))
        bw = ctx.enter_context(tc.tile_pool(name="bw", bufs=1))
        p1 = ctx.enter_context(tc.tile_pool(name="p1", bufs=3, space="PSUM"))
        p2 = ctx.enter_context(tc.tile_pool(name="p2", bufs=1, space="PSUM"))
        p3 = ctx.enter_context(tc.tile_pool(name="p3", bufs=1, space="PSUM"))

        warmt = cw.tile([128, 512], F16)
        nc.gpsimd.memset(warmt[:], 0.0)
        wpsum = p2.tile([128, 512], F32, tag="prh", name="warmps")
        for wi in range(7):
            nc.tensor.matmul(wpsum[:], lhsT=warmt[:, 0:128], rhs=warmt[:],
                             start=True, stop=True)
        w1all = cw.tile([128, 12 * 128], F16); nc.sync.dma_start(w1all[:], w1all_d)
        small = cw.tile([128, 8], F32); nc.sync.dma_start(small[:], small_d)
        l2w = cw.tile([128, 256], F8)
        pw = cw.tile([66, 64], F16)
        pw2 = cw.tile([65, 2], F16)
        ones4 = cw.tile([128, 32], F16)
        xb = cw.tile([128, 2 * BARC], F32)

        # barrier working tiles (fp32 until the neighbor reduce)
        sqx = bw.tile([128, BARC], F32)
        sqy = bw.tile([128, BARC], F32)
        n2 = bw.tile([128, BARC], F32)
        sr = bw.tile([128, BARC], F32)
        dd = bw.tile([128, BARC], F32)
        rr = bw.tile([128, BARC], F32)
        rpx = bw.tile([128, BARC], F16)
        rpy = bw.tile([128, BARC], F16)
        barS = bw.tile([32, 1024], F32)
        xbx = xb[:, 0:BARC]
        xby = xb[:, BARC:2 * BARC]

        def bar_step(i):
            if i == 0:
                nc.gpsimd.tensor_tensor(sqx[:], xbx, xbx, op=ALU.mult)
            elif i == 1:
                nc.gpsimd.tensor_tensor(sqy[:], xby, xby, op=ALU.mult)
            elif i == 2:
                nc.gpsimd.tensor_tensor(n2[:], sqx[:], sqy[:], op=ALU.add)
            elif i == 3:
                nc.scalar.activation(sr[:], n2[:], AF.Sqrt)
            elif i == 4:
                nc.gpsimd.tensor_scalar(dd[:], sr[:], 1.0 / B_GAMMA,
                                         -DS / B_GAMMA, op0=ALU.mult, op1=ALU.add)
            elif i == 5:
                nc.vector.reciprocal_approx_fast(out=rr[:], in_=dd[:])
            elif i == 6:
                nc.gpsimd.tensor_tensor(rpx[:], xbx, rr[:], op=ALU.mult)
            elif i == 7:
                nc.gpsimd.tensor_tensor(rpy[:], xby, rr[:], op=ALU.mult)
            elif i == 8:
                bx = p3.tile([32, 400], F32, tag="pb", name="barxp")
                for f in range(4):
                    nc.tensor.matmul(bx[:], lhsT=ones4[:],
                                     rhs=rpx[:, 400 * f:400 * f + 400],
                                     start=(f == 0), stop=(f == 3))
                nc.vector.tensor_copy(barS[:, 0:400], bx[:])
            elif i == 9:
                by = p3.tile([32, 400], F32, tag="pb", name="baryp")
                for f in range(4):
                    nc.tensor.matmul(by[:], lhsT=ones4[:],
                                     rhs=rpy[:, 400 * f:400 * f + 400],
                                     start=(f == 0), stop=(f == 3))
                nc.vector.tensor_copy(barS[:, 512:912], by[:])

        peacc = [None, None]

        def chunk_tail(c):
            et = pep.tile([32, 512], F32, tag="et")
            nc.vector.transpose(et[:], peacc[c][:])
            e1 = pep.tile([32, 512], F32, tag="e1")
            nc.scalar.activation(e1[:], et[:], AF.Tanh)
            act = pep.tile([32, 512], F32, tag="act")
            actv = act[:].rearrange("p (b m u) -> p b m u", m=16, u=2)
            e1v = e1[:].rearrange("p (b m u) -> p b m u", m=16, u=2)
            bxv = barS[:, 256 * c:256 * c + 256].rearrange(
                "p (m b o) -> p b m o", b=16, o=1)
            byv = barS[:, 512 + 256 * c:512 + 256 * c + 256].rearrange(
                "p (m b o) -> p b m o", b=16, o=1)
            nc.vector.tensor_add(actv[:, :, :, 0:1], e1v[:, :, :, 0:1], bxv)
            nc.vector.tensor_add(actv[:, :, :, 1:2], e1v[:, :, :, 1:2], byv)
            yt = pep.tile([32, 512], F32, tag="yt")
            nc.scalar.activation(yt[:], act[:], AF.Tanh)
            y2 = pep.tile([32, 512], F32, tag="y2")
            nc.vector.tensor_scalar_mul(y2[:], yt[:], 2.0)
            nc.sync.dma_start(y_d[:, 512 * c:512 * c + 512], y2[:])

        # 3-stage software pipeline: iteration i runs layer-1(i) interleaved
        # with the fused layer-2 DR matmuls of group i-1 and the psi head of
        # group i-2, so TensorE never waits on a just-issued evac.
        st1 = None   # group i-1 state
        st2 = None   # group i-2 state
        st3 = None   # group i-3 state
        for i in range(G512 + 3):
            cur = None
            if i == 0:
                nc.sync.dma_start(l2w[:], l2w_d)
                nc.sync.dma_start(pw[:], pw_d)
                nc.sync.dma_start(pw2[:], pw2_d)
                nc.sync.dma_start(ones4[:], ones4_d)
            if i == 2:
                nc.sync.dma_start(xb[:], xb_d)
            if i < G512:
                cs = i * 512
                xt = xin.tile([128, 512], F16, tag="xt", name=f"xt{i}")
                nc.sync.dma_start(xt[:], xt_d[:, cs:cs + 512])
                rg = rgp.tile([66, 512], F16, tag="rg", bufs=3, name=f"rg{i}")
                nc.sync.dma_start(rg[64:66, :], gg_d[:, cs:cs + 512])
                parents = [s8p.tile([128, 1024], F8, tag=f"s{j}",
                                    name=f"s8_{i}_{j}") for j in range(6)]
                cur = dict(i=i, xt=xt, rg=rg, parents=parents)

            if st2 is not None:
                g2 = st2["i"]
                pb = p3.tile([128, 512], F32, tag="pb", name=f"pb{g2}")
                st2["pb"] = pb
                nc.tensor.matmul(pb[0:64, :], lhsT=pw[:], rhs=st2["rg"][:],
                                 start=True, stop=True)

            if st1 is not None:
                prh = p2.tile([64, 512], F32, tag="prh", name=f"prh{st1['i']}")
                st1["prh"] = prh

            def l1mm(idx):
                sp = idx % 4
                j, ko = idx // 2, idx % 2
                if ko == 0:
                    cur["ps%d" % j] = p1.tile([128, 1024], F32, tag="ps",
                                              name=f"ps_{i}_{j}")
                ps = cur["ps%d" % j]
                nc.tensor.matmul(ps[:, ko * 512:ko * 512 + 512],
                                 lhsT=w1all[32 * sp:32 * sp + 32,
                                            idx * 128:idx * 128 + 128],
                                 rhs=cur["xt"][32 * sp:32 * sp + 32, :],
                                 start=True, stop=True,
                                 tile_position=(32 * sp, 0))

            def evac(j):
                bcol = small[:, 0:1] if j < 4 else small[:, 1:2]
                dst = cur["parents"][j][:]
                ps = cur["ps%d" % j]
                if j % 2 == 0:
                    nc.scalar.activation(dst, ps[:], AF.Relu, bias=bcol)
                else:
                    nc.vector.tensor_scalar(dst, ps[:], bcol, 0.0,
                                            op0=ALU.add, op1=ALU.max)

            def drmm(j):
                rhs = st1["parents"][j][:].rearrange("p (ko n) -> p ko n", ko=2)
                wsel = l2w[:, 0:128] if j < 4 else l2w[:, 128:256]
                lhsT = wsel.rearrange("p (ko q) -> p ko q", ko=2)
                nc.tensor.matmul(st1["prh"][:], lhsT=lhsT, rhs=rhs,
                                 start=(j == 0), stop=(j == 5),
                                 perf_mode=PM.DoubleRow)

            seq = [('m', 0), ('m', 1), ('e', 0), ('m', 2), ('m', 3), ('e', 1),
                   ('m', 4), ('m', 5), ('e', 2), ('d', 0), ('d', 1),
                   ('m', 6), ('m', 7), ('e', 3), ('d', 2), ('d', 3),
                   ('m', 8), ('m', 9), ('e', 4), ('d', 4),
                   ('m', 10), ('m', 11), ('e', 5), ('d', 5)]
            for kind, a in seq:
                if kind in ('m', 'e'):
                    if cur is None:
                        continue
                    (l1mm if kind == 'm' else evac)(a)
                else:
                    if st1 is None:
                        continue
                    drmm(a)

            if st1 is not None:
                nc.scalar.activation(st1["rg"][0:64, :], st1["prh"][:], AF.Relu,
                                     bias=small[0:64, 2:3], scale=1.0 / (S1 * S2))
            if st2 is not None:
                phh = rgp.tile([65, 512], F16, tag="phh", name=f"phh{g2}")
                nc.gpsimd.memset(phh[64:65, :], 1.0)
                nc.scalar.activation(phh[0:64, :], st2["pb"][0:64, :], AF.Relu,
                                     bias=small[0:64, 3:4])
                nc.tensor.matmul(st2["pb"][64:66, :], lhsT=pw2[:], rhs=phh[:],
                                 start=True, stop=True, tile_position=(0, 64))
                m2, c2 = g2 % 16, g2 // 16
                if m2 == 0:
                    peacc[c2] = pep.tile([32, 512], F32, tag="pe",
                                         name=f"peacc{c2}")
                petmp = rgp.tile([2, 512], F32, tag="petmp", name=f"petmp{g2}")
                nc.vector.tensor_copy(petmp[:], st2["pb"][64:66, :])
                nc.sync.dma_start(peacc[c2][2 * m2:2 * m2 + 2, :], petmp[:])

            if i < 10:
                bar_step(i)
            if st2 is not None and st2["i"] == 15:
                chunk_tail(0)
            st3, st2, st1 = st2, st1, cur
        chunk_tail(1)
    return nc


_CACHED = {}


def kernel(**inputs):
    x = np.asarray(inputs["x"], np.float32)
    wk = _pack_weights(**{k: np.asarray(v, np.float32) for k, v in inputs.items()
                          if k != "x"})
    in_maps = []
    for core in range(NCORE):
        xp = np.zeros((AP_, D_OBS), np.float32)
        xp[:AC] = x[core * AC:(core + 1) * AC]
        xt = np.zeros((128, AP_), np.float16)
        for idx in range(8):
            sp, wv = idx % 4, idx // 4
            for f in range(8):
                n, dd_ = 2 * idx + f // 4, f % 4
                xt[32 * sp + 8 * wv + f] = xp[:, 5 + 4 * n + dd_].astype(np.float16)
        for q in range(4):
            for t in range(4):
                o, dd_ = 2 * q + t // 2, t % 2
                xt[32 * q + 16 + t] = xp[:, 69 + 2 * o + dd_].astype(np.float16)
        gg = np.ascontiguousarray(xp[:, 0:2].T.astype(np.float16))
        p = -xp[:, 5:69].reshape(AP_, 16, 4)[:, :, 0:2]       # [A, 16, 2]
        # [gi, bj, a32, nhi, nlo] -> partition 32*nlo + a32, col (gi*16+bj)*4+nhi
        pr = p.reshape(G512, 16, 32, 4, 4, 2)
        xbh = pr.transpose(5, 4, 2, 3, 0, 1).reshape(2, 128, BARC)
        xb = np.ascontiguousarray(
            np.concatenate([xbh[0], xbh[1]], axis=1).astype(np.float32))
        m = dict(wk)
        m["xt"] = np.ascontiguousarray(xt)
        m["gg"] = gg
        m["xb"] = xb
        in_maps.append(m)

    if "nc" not in _CACHED:
        nc = bacc.Bacc("TRN2", target_bir_lowering=False, debug=False,
                       num_devices=NCORE)
        _build(nc)
        nc.compile()
        _CACHED["nc"] = nc
    nc = _CACHED["nc"]
    res = run_bass_kernel_spmd(nc, in_maps, core_ids=list(range(NCORE)))
    _CACHED["last_res"] = res
    out = np.empty((B, ADIM), np.float32)
    for core in range(NCORE):
        Y = res.results[core]["y"]                            # [32, 1024]
        Y5 = Y.reshape(32, 2, 16, 16, 2).transpose(1, 3, 2, 0, 4)
        Y5 = Y5.reshape(32, 512, 2)[:G512].reshape(AP_, 2)
        out[core * AC:(core + 1) * AC] = Y5[:AC]
    return out


if __name__ == "__main__":
    import reference
    ins = {k: np.asarray(v) for k, v in reference.setup_inputs().items()}
    got = kernel(**ins)
    exp = np.asarray(reference.reference(**ins))
    err = np.abs(got - exp).max()
    rel = err / np.abs(exp).max()
    print(f"absmax {err:.4e} rel {rel:.4e}")


# revision 25
# speedup vs baseline: 1.0040x; 1.0040x over previous
"""Barrier_Net TRN2 kernel: 8-core data-parallel Bass/Tile implementation (v2).

Per-core structure (12800 padded agents = 25 groups x 512):
  - Layer-1 MLPs: 12 fp16 matmuls/group (weights pre-scaled x32), relu evac
    to fp8e4 (values 32*relu) split across ScalarE/VectorE.
  - Layer-2 + rho1 fused by linearity (lhsT = (w2 @ rho_w1) * 8 in fp8),
    6 DoubleRow fp8 matmuls/group (contract 256) accumulate rho hidden
    pre-act (x256) in one PSUM tile.
  - rho2 + psi1 fused (lhsT rows 0:64 = rho_w2 @ psi_w1[:2] / 256), g rows
    appended to the same rhs tile -> single matmul.
  - empty-head e stashed per group into a [32,512] accumulator (16 groups),
    transposed once per chunk, tanh'd in batch (one act-table switch total).
  - barrier computed fully in fp32 (accuracy-critical) in a [128,1600]x2
    layout, neighbor reduce on DVE + partition fold via matmul with ones.
"""
import sys, os
sys.path.insert(0, "/opt/trn_rl_repo")
import numpy as np
import ml_dtypes
import concourse.bacc as bacc
import concourse.tile as tile
import concourse.mybir as mybir
from concourse.bass_utils import run_bass_kernel_spmd
from contextlib import ExitStack

F32 = mybir.dt.float32
F16 = mybir.dt.float16
F8 = mybir.dt.float8e4
AF = mybir.ActivationFunctionType
ALU = mybir.AluOpType
PM = mybir.MatmulPerfMode

B, NN, NO, SD = 100000, 16, 8, 4
H, PHI_OUT, ADIM = 64, 16, 2
DS, B_GAMMA = 0.2, 0.01
D_OBS = 85
NCORE = 8
AC = B // NCORE            # 12500 agents per core
G512 = 25                  # groups of 512
AP_ = G512 * 512           # padded agents per core = 12800
S1 = 32.0                  # layer-1 weight prescale
S2 = 8.0                   # fused layer-2 weight prescale
BARC = G512 * 16 * 4       # 1600 barrier cols per half

SCALAR_EVACS = frozenset({0, 1, 3, 4, 6, 7, 9, 10})


def _pack_weights(phi_w1, phi_b1, phi_w2, phi_b2, obs_w1, obs_b1, obs_w2, obs_b2,
                  rho_w1, rho_b1, rho_w2, rho_b2, psi_w1, psi_b1, psi_w2, psi_b2):
    # layer-1 block-diagonal lhsT, 2 elements per matmul, prescaled x32
    W1P = np.zeros((8, 128, 128), np.float32)
    for k in range(8):
        for j in range(2):
            n = 2 * k + j
            W1P[k, 5 + 4 * n:5 + 4 * n + 4, 64 * j:64 * j + 64] = phi_w1 * S1
    OW1P = np.zeros((4, 128, 128), np.float32)
    for m in range(4):
        for j in range(2):
            o = 2 * m + j
            OW1P[m, 69 + 2 * o:69 + 2 * o + 2, 64 * j:64 * j + 64] = obs_w1 * S1
    # fused layer-2 + rho1 (DoubleRow lhsT [128, ko=2, 64]), x8
    W2R = np.tile((phi_w2 @ rho_w1) * S2, (2, 1))       # [128, 64]
    OW2R = np.tile((obs_w2 @ rho_w1) * S2, (2, 1))
    l2w = np.zeros((128, 2, 2, 64), np.float32)          # [p, which, ko, j]
    l2w[:, 0, 0] = l2w[:, 0, 1] = W2R
    l2w[:, 1, 0] = l2w[:, 1, 1] = OW2R
    # fused rho2+psi1 lhsT [66, 64]; rh carries a 1/(S1*S2) scale already
    PW = np.zeros((66, 64), np.float32)
    PW[0:64] = rho_w2 @ psi_w1[0:2]
    PW[64:66] = psi_w1[2:4]
    # bias columns
    RB1C = (rho_b1 + (NN * phi_b2 + NO * obs_b2) @ rho_w1).reshape(64, 1)
    PB1C = (psi_b1 + rho_b2 @ psi_w1[0:2]).reshape(64, 1)
    PB2C = psi_b2.reshape(2, 1)
    small = np.zeros((128, 8), np.float32)
    small[:, 0:1] = np.tile(phi_b1, 2).reshape(128, 1) * S1
    small[:, 1:2] = np.tile(obs_b1, 2).reshape(128, 1) * S1
    small[0:64, 2:3] = RB1C
    small[0:64, 3:4] = PB1C
    small[0:2, 4:5] = PB2C
    ones4 = np.zeros((128, 32), np.float32)
    for p in range(128):
        ones4[p, p % 32] = 1.0
    f8 = ml_dtypes.float8_e4m3
    PW2B = np.concatenate([psi_w2, psi_b2.reshape(1, 2)], axis=0)
    return {
        "w1p": W1P.transpose(1, 0, 2).reshape(128, 8 * 128).astype(np.float16),
        "ow1p": OW1P.transpose(1, 0, 2).reshape(128, 4 * 128).astype(np.float16),
        "l2w": l2w.reshape(128, 256).astype(f8),
        "pw": PW.astype(np.float16),
        "pw2": PW2B.astype(np.float16),
        "ones4": ones4.astype(np.float16),
        "small": small.astype(np.float32),
    }


def _build(nc):
    xt_d = nc.dram_tensor("xt", [128, AP_], F16, kind="ExternalInput").ap()
    gg_d = nc.dram_tensor("gg", [2, AP_], F16, kind="ExternalInput").ap()
    xb_d = nc.dram_tensor("xb", [128, 2 * BARC], F32, kind="ExternalInput").ap()
    w1p_d = nc.dram_tensor("w1p", [128, 8 * 128], F16, kind="ExternalInput").ap()
    ow1p_d = nc.dram_tensor("ow1p", [128, 4 * 128], F16, kind="ExternalInput").ap()
    l2w_d = nc.dram_tensor("l2w", [128, 256], F8, kind="ExternalInput").ap()
    pw_d = nc.dram_tensor("pw", [66, 64], F16, kind="ExternalInput").ap()
    pw2_d = nc.dram_tensor("pw2", [65, 2], F16, kind="ExternalInput").ap()
    ones4_d = nc.dram_tensor("ones4", [128, 32], F16, kind="ExternalInput").ap()
    small_d = nc.dram_tensor("small", [128, 8], F32, kind="ExternalInput").ap()
    y_d = nc.dram_tensor("y", [32, 1024], F32, kind="ExternalOutput").ap()

    with tile.TileContext(nc) as tc, ExitStack() as ctx, \
            nc.allow_low_precision(reason="fp16 barrier partials validated"):
        cw = ctx.enter_context(tc.tile_pool(name="cw", bufs=1))
        xin = ctx.enter_context(tc.tile_pool(name="xin", bufs=3))
        s8p = ctx.enter_context(tc.tile_pool(name="s8p", bufs=2))
        rgp = ctx.enter_context(tc.tile_pool(name="rgp", bufs=2))
        pep = ctx.enter_context(tc.tile_pool(name="pep", bufs=2))
        bw = ctx.enter_context(tc.tile_pool(name="bw", bufs=1))
        p1 = ctx.enter_context(tc.tile_pool(name="p1", bufs=3, space="PSUM"))
        p2 = ctx.enter_context(tc.tile_pool(name="p2", bufs=1, space="PSUM"))
        p3 = ctx.enter_context(tc.tile_pool(name="p3", bufs=1, space="PSUM"))

        warmt = cw.tile([128, 512], F16)
        nc.gpsimd.memset(warmt[:], 0.0)
        wpsum = p2.tile([128, 512], F32, tag="prh", name="warmps")
        for wi in range(7):
            nc.tensor.matmul(wpsum[:], lhsT=warmt[:, 0:128], rhs=warmt[:],
                             start=True, stop=True)
        w1p = cw.tile([128, 8 * 128], F16); nc.sync.dma_start(w1p[:], w1p_d)
        small = cw.tile([128, 8], F32); nc.sync.dma_start(small[:], small_d)
        ow1p = cw.tile([128, 4 * 128], F16); nc.sync.dma_start(ow1p[:], ow1p_d)
        l2w = cw.tile([128, 256], F8)
        pw = cw.tile([66, 64], F16)
        pw2 = cw.tile([65, 2], F16)
        ones4 = cw.tile([128, 32], F16)
        xb = cw.tile([128, 2 * BARC], F32)

        # barrier working tiles (fp32 until the neighbor reduce)
        sqx = bw.tile([128, BARC], F32)
        sqy = bw.tile([128, BARC], F32)
        n2 = bw.tile([128, BARC], F32)
        sr = bw.tile([128, BARC], F32)
        dd = bw.tile([128, BARC], F32)
        rr = bw.tile([128, BARC], F32)
        rpx = bw.tile([128, BARC], F16)
        rpy = bw.tile([128, BARC], F16)
        barS = bw.tile([32, 1024], F32)
        xbx = xb[:, 0:BARC]
        xby = xb[:, BARC:2 * BARC]

        def bar_step(i):
            if i == 0:
                nc.gpsimd.tensor_tensor(sqx[:], xbx, xbx, op=ALU.mult)
            elif i == 1:
                nc.gpsimd.tensor_tensor(sqy[:], xby, xby, op=ALU.mult)
            elif i == 2:
                nc.gpsimd.tensor_tensor(n2[:], sqx[:], sqy[:], op=ALU.add)
            elif i == 3:
                nc.scalar.activation(sr[:], n2[:], AF.Sqrt)
            elif i == 4:
                nc.gpsimd.tensor_scalar(dd[:], sr[:], 1.0 / B_GAMMA,
                                         -DS / B_GAMMA, op0=ALU.mult, op1=ALU.add)
            elif i == 5:
                nc.vector.reciprocal_approx_fast(out=rr[:], in_=dd[:])
            elif i == 6:
                nc.gpsimd.tensor_tensor(rpx[:], xbx, rr[:], op=ALU.mult)
            elif i == 7:
                nc.gpsimd.tensor_tensor(rpy[:], xby, rr[:], op=ALU.mult)
            elif i == 8:
                bx = p3.tile([32, 400], F32, tag="pb", name="barxp")
                for f in range(4):
                    nc.tensor.matmul(bx[:], lhsT=ones4[:],
                                     rhs=rpx[:, 400 * f:400 * f + 400],
                                     start=(f == 0), stop=(f == 3))
                nc.vector.tensor_copy(barS[:, 0:400], bx[:])
            elif i == 9:
                by = p3.tile([32, 400], F32, tag="pb", name="baryp")
                for f in range(4):
                    nc.tensor.matmul(by[:], lhsT=ones4[:],
                                     rhs=rpy[:, 400 * f:400 * f + 400],
                                     start=(f == 0), stop=(f == 3))
                nc.vector.tensor_copy(barS[:, 512:912], by[:])

        peacc = [None, None]

        def chunk_tail(c):
            et = pep.tile([32, 512], F32, tag="et")
            nc.vector.transpose(et[:], peacc[c][:])
            e1 = pep.tile([32, 512], F32, tag="e1")
            nc.scalar.activation(e1[:], et[:], AF.Tanh)
            act = pep.tile([32, 512], F32, tag="act")
            actv = act[:].rearrange("p (b m u) -> p b m u", m=16, u=2)
            e1v = e1[:].rearrange("p (b m u) -> p b m u", m=16, u=2)
            bxv = barS[:, 256 * c:256 * c + 256].rearrange(
                "p (m b o) -> p b m o", b=16, o=1)
            byv = barS[:, 512 + 256 * c:512 + 256 * c + 256].rearrange(
                "p (m b o) -> p b m o", b=16, o=1)
            nc.vector.tensor_add(actv[:, :, :, 0:1], e1v[:, :, :, 0:1], bxv)
            nc.vector.tensor_add(actv[:, :, :, 1:2], e1v[:, :, :, 1:2], byv)
            yt = pep.tile([32, 512], F32, tag="yt")
            nc.scalar.activation(yt[:], act[:], AF.Tanh)
            y2 = pep.tile([32, 512], F32, tag="y2")
            nc.vector.tensor_scalar_mul(y2[:], yt[:], 2.0)
            nc.sync.dma_start(y_d[:, 512 * c:512 * c + 512], y2[:])

        # 3-stage software pipeline: iteration i runs layer-1(i) interleaved
        # with the fused layer-2 DR matmuls of group i-1 and the psi head of
        # group i-2, so TensorE never waits on a just-issued evac.
        st1 = None   # group i-1 state
        st2 = None   # group i-2 state
        st3 = None   # group i-3 state
        for i in range(G512 + 3):
            cur = None
            if i == 0:
                nc.sync.dma_start(l2w[:], l2w_d)
                nc.sync.dma_start(pw[:], pw_d)
                nc.sync.dma_start(pw2[:], pw2_d)
                nc.sync.dma_start(ones4[:], ones4_d)
            if i == 2:
                nc.sync.dma_start(xb[:], xb_d)
            if i < G512:
                cs = i * 512
                xt = xin.tile([128, 512], F16, tag="xt", name=f"xt{i}")
                nc.sync.dma_start(xt[:], xt_d[:, cs:cs + 512])
                rg = rgp.tile([66, 512], F16, tag="rg", bufs=3, name=f"rg{i}")
                nc.sync.dma_start(rg[64:66, :], gg_d[:, cs:cs + 512])
                parents = [s8p.tile([128, 1024], F8, tag=f"s{j}",
                                    name=f"s8_{i}_{j}") for j in range(6)]
                cur = dict(i=i, xt=xt, rg=rg, parents=parents)

            if st2 is not None:
                g2 = st2["i"]
                pb = p3.tile([128, 512], F32, tag="pb", name=f"pb{g2}")
                st2["pb"] = pb
                nc.tensor.matmul(pb[0:64, :], lhsT=pw[:], rhs=st2["rg"][:],
                                 start=True, stop=True)

            if st1 is not None:
                prh = p2.tile([64, 512], F32, tag="prh", name=f"prh{st1['i']}")
                st1["prh"] = prh

            def l1mm(idx):
                j, ko = idx // 2, idx % 2
                if ko == 0:
                    cur["ps%d" % j] = p1.tile([128, 1024], F32, tag="ps",
                                              name=f"ps_{i}_{j}")
                ps = cur["ps%d" % j]
                if idx < 8:
                    lhsT = w1p[:, idx * 128:idx * 128 + 128]
                else:
                    lhsT = ow1p[:, (idx - 8) * 128:(idx - 8) * 128 + 128]
                nc.tensor.matmul(ps[:, ko * 512:ko * 512 + 512],
                                 lhsT=lhsT, rhs=cur["xt"][:],
                                 start=True, stop=True)

            def evac(j):
                bcol = small[:, 0:1] if j < 4 else small[:, 1:2]
                dst = cur["parents"][j][:]
                ps = cur["ps%d" % j]
                if j % 2 == 0:
                    nc.scalar.activation(dst, ps[:], AF.Relu, bias=bcol)
                else:
                    nc.vector.tensor_scalar(dst, ps[:], bcol, 0.0,
                                            op0=ALU.add, op1=ALU.max)

            def drmm(j):
                rhs = st1["parents"][j][:].rearrange("p (ko n) -> p ko n", ko=2)
                wsel = l2w[:, 0:128] if j < 4 else l2w[:, 128:256]
                lhsT = wsel.rearrange("p (ko q) -> p ko q", ko=2)
                nc.tensor.matmul(st1["prh"][:], lhsT=lhsT, rhs=rhs,
                                 start=(j == 0), stop=(j == 5),
                                 perf_mode=PM.DoubleRow)

            seq = [('m', 0), ('m', 1), ('e', 0), ('m', 2), ('m', 3), ('e', 1),
                   ('m', 4), ('m', 5), ('e', 2), ('d', 0), ('d', 1),
                   ('m', 6), ('m', 7), ('e', 3), ('d', 2), ('d', 3),
                   ('m', 8), ('m', 9), ('e', 4), ('d', 4),
                   ('m', 10), ('m', 11), ('e', 5), ('d', 5)]
            for kind, a in seq:
                if kind in ('m', 'e'):
                    if cur is None:
                        continue
                    (l1mm if kind == 'm' else evac)(a)
                else:
                    if st1 is None:
                        continue
                    drmm(a)

            if st1 is not None:
                nc.scalar.activation(st1["rg"][0:64, :], st1["prh"][:], AF.Relu,
                                     bias=small[0:64, 2:3], scale=1.0 / (S1 * S2))
            if st2 is not None:
                phh = rgp.tile([65, 512], F16, tag="phh", name=f"phh{g2}")
                nc.gpsimd.memset(phh[64:65, :], 1.0)
                nc.scalar.activation(phh[0:64, :], st2["pb"][0:64, :], AF.Relu,
                                     bias=small[0:64, 3:4])
                nc.tensor.matmul(st2["pb"][64:66, :], lhsT=pw2[:], rhs=phh[:],
                                 start=True, stop=True, tile_position=(0, 64))
                m2, c2 = g2 % 16, g2 // 16
                if m2 == 0:
                    peacc[c2] = pep.tile([32, 512], F32, tag="pe",
                                         name=f"peacc{c2}")
                petmp = rgp.tile([2, 512], F32, tag="petmp", name=f"petmp{g2}")
                nc.vector.tensor_copy(petmp[:], st2["pb"][64:66, :])
                nc.sync.dma_start(peacc[c2][2 * m2:2 * m2 + 2, :], petmp[:])

            if i < 10:
                bar_step(i)
            if st2 is not None and st2["i"] == 15:
                chunk_tail(0)
            st3, st2, st1 = st2, st1, cur
        chunk_tail(1)
    return nc


_CACHED = {}


def kernel(**inputs):
    x = np.asarray(inputs["x"], np.float32)
    wk = _pack_weights(**{k: np.asarray(v, np.float32) for k, v in inputs.items()
                          if k != "x"})
    in_maps = []
    for core in range(NCORE):
        xp = np.zeros((AP_, D_OBS), np.float32)
        xp[:AC] = x[core * AC:(core + 1) * AC]
        xt = np.zeros((128, AP_), np.float16)
        xt[0:D_OBS] = xp.T.astype(np.float16)
        gg = np.ascontiguousarray(xp[:, 0:2].T.astype(np.float16))
        p = -xp[:, 5:69].reshape(AP_, 16, 4)[:, :, 0:2]       # [A, 16, 2]
        # [gi, bj, a32, nhi, nlo] -> partition 32*nlo + a32, col (gi*16+bj)*4+nhi
        pr = p.reshape(G512, 16, 32, 4, 4, 2)
        xbh = pr.transpose(5, 4, 2, 3, 0, 1).reshape(2, 128, BARC)
        xb = np.ascontiguousarray(
            np.concatenate([xbh[0], xbh[1]], axis=1).astype(np.float32))
        m = dict(wk)
        m["xt"] = np.ascontiguousarray(xt)
        m["gg"] = gg
        m["xb"] = xb
        in_maps.append(m)

    if "nc" not in _CACHED:
        nc = bacc.Bacc("TRN2", target_bir_lowering=False, debug=False,
                       num_devices=NCORE)
        _build(nc)
        nc.compile()
        _CACHED["nc"] = nc
    nc = _CACHED["nc"]
    res = run_bass_kernel_spmd(nc, in_maps, core_ids=list(range(NCORE)))
    _CACHED["last_res"] = res
    out = np.empty((B, ADIM), np.float32)
    for core in range(NCORE):
        Y = res.results[core]["y"]                            # [32, 1024]
        Y5 = Y.reshape(32, 2, 16, 16, 2).transpose(1, 3, 2, 0, 4)
        Y5 = Y5.reshape(32, 512, 2)[:G512].reshape(AP_, 2)
        out[core * AC:(core + 1) * AC] = Y5[:AC]
    return out


if __name__ == "__main__":
    import reference
    ins = {k: np.asarray(v) for k, v in reference.setup_inputs().items()}
    got = kernel(**ins)
    exp = np.asarray(reference.reference(**ins))
    err = np.abs(got - exp).max()
    rel = err / np.abs(exp).max()
    print(f"absmax {err:.4e} rel {rel:.4e}")


# revision 26
# speedup vs baseline: 1.1957x; 1.1910x over previous
"""Barrier_Net TRN2 kernel: 8-core data-parallel Bass/Tile implementation (v2).

Per-core structure (12800 padded agents = 25 groups x 512):
  - Layer-1 MLPs: 12 fp16 matmuls/group (weights pre-scaled x32), relu evac
    to fp8e4 (values 32*relu) split across ScalarE/VectorE.
  - Layer-2 + rho1 fused by linearity (lhsT = (w2 @ rho_w1) * 8 in fp8),
    6 DoubleRow fp8 matmuls/group (contract 256) accumulate rho hidden
    pre-act (x256) in one PSUM tile.
  - rho2 + psi1 fused (lhsT rows 0:64 = rho_w2 @ psi_w1[:2] / 256), g rows
    appended to the same rhs tile -> single matmul.
  - empty-head e stashed per group into a [32,512] accumulator (16 groups),
    transposed once per chunk, tanh'd in batch (one act-table switch total).
  - barrier computed fully in fp32 (accuracy-critical) in a [128,1600]x2
    layout, neighbor reduce on DVE + partition fold via matmul with ones.
"""
import sys, os
sys.path.insert(0, "/opt/trn_rl_repo")
import numpy as np
import ml_dtypes
import concourse.bacc as bacc
import concourse.tile as tile
import concourse.mybir as mybir
from concourse.bass_utils import run_bass_kernel_spmd
from contextlib import ExitStack

F32 = mybir.dt.float32
F16 = mybir.dt.float16
F8 = mybir.dt.float8e4
AF = mybir.ActivationFunctionType
ALU = mybir.AluOpType
PM = mybir.MatmulPerfMode

B, NN, NO, SD = 100000, 16, 8, 4
H, PHI_OUT, ADIM = 64, 16, 2
DS, B_GAMMA = 0.2, 0.01
D_OBS = 85
NCORE = 8
AC = B // NCORE            # 12500 agents per core
G512 = 25                  # groups of 512
AP_ = G512 * 512           # padded agents per core = 12800
S1 = 32.0                  # layer-1 weight prescale
S2 = 8.0                   # fused layer-2 weight prescale
BARC = G512 * 16 * 4       # 1600 barrier cols per half

SCALAR_EVACS = frozenset({0, 1, 3, 4, 6, 7, 9, 10})


def _pack_weights(phi_w1, phi_b1, phi_w2, phi_b2, obs_w1, obs_b1, obs_w2, obs_b2,
                  rho_w1, rho_b1, rho_w2, rho_b2, psi_w1, psi_b1, psi_w2, psi_b2):
    # layer-1 block-diagonal lhsT, 2 elements per matmul, prescaled x32
    W1P = np.zeros((8, 128, 128), np.float32)
    for k in range(8):
        for j in range(2):
            n = 2 * k + j
            W1P[k, 5 + 4 * n:5 + 4 * n + 4, 64 * j:64 * j + 64] = phi_w1 * S1
    OW1P = np.zeros((4, 128, 128), np.float32)
    for m in range(4):
        for j in range(2):
            o = 2 * m + j
            OW1P[m, 69 + 2 * o:69 + 2 * o + 2, 64 * j:64 * j + 64] = obs_w1 * S1
    # fused layer-2 + rho1 (DoubleRow lhsT [128, ko=2, 64]), x8
    W2R = np.tile((phi_w2 @ rho_w1) * S2, (2, 1))       # [128, 64]
    OW2R = np.tile((obs_w2 @ rho_w1) * S2, (2, 1))
    l2w = np.zeros((128, 2, 2, 64), np.float32)          # [p, which, ko, j]
    l2w[:, 0, 0] = l2w[:, 0, 1] = W2R
    l2w[:, 1, 0] = l2w[:, 1, 1] = OW2R
    # fused rho2+psi1 lhsT [66, 64]; rh carries a 1/(S1*S2) scale already
    PW = np.zeros((66, 64), np.float32)
    PW[0:64] = rho_w2 @ psi_w1[0:2]
    PW[64:66] = psi_w1[2:4]
    # bias columns
    RB1C = (rho_b1 + (NN * phi_b2 + NO * obs_b2) @ rho_w1).reshape(64, 1)
    PB1C = (psi_b1 + rho_b2 @ psi_w1[0:2]).reshape(64, 1)
    PB2C = psi_b2.reshape(2, 1)
    small = np.zeros((128, 8), np.float32)
    small[:, 0:1] = np.tile(phi_b1, 2).reshape(128, 1) * S1
    small[:, 1:2] = np.tile(obs_b1, 2).reshape(128, 1) * S1
    small[0:64, 2:3] = RB1C
    small[0:64, 3:4] = PB1C
    small[0:2, 4:5] = PB2C
    ones4 = np.zeros((128, 32), np.float32)
    for p in range(128):
        ones4[p, p % 32] = 1.0
    f8 = ml_dtypes.float8_e4m3
    PW2B = np.concatenate([psi_w2, psi_b2.reshape(1, 2)], axis=0)
    return {
        "w1p": W1P.transpose(1, 0, 2).reshape(128, 8 * 128).astype(np.float16),
        "ow1p": OW1P.transpose(1, 0, 2).reshape(128, 4 * 128).astype(np.float16),
        "l2w": l2w.reshape(128, 256).astype(f8),
        "pw": PW.astype(np.float16),
        "pw2": PW2B.astype(np.float16),
        "ones4": ones4.astype(np.float16),
        "small": small.astype(np.float32),
    }


def _build(nc):
    xt_d = nc.dram_tensor("xt", [128, AP_], F16, kind="ExternalInput").ap()
    gg_d = nc.dram_tensor("gg", [2, AP_], F16, kind="ExternalInput").ap()
    xb_d = nc.dram_tensor("xb", [128, 2 * BARC], F32, kind="ExternalInput").ap()
    w1p_d = nc.dram_tensor("w1p", [128, 8 * 128], F16, kind="ExternalInput").ap()
    ow1p_d = nc.dram_tensor("ow1p", [128, 4 * 128], F16, kind="ExternalInput").ap()
    l2w_d = nc.dram_tensor("l2w", [128, 256], F8, kind="ExternalInput").ap()
    pw_d = nc.dram_tensor("pw", [66, 64], F16, kind="ExternalInput").ap()
    pw2_d = nc.dram_tensor("pw2", [65, 2], F16, kind="ExternalInput").ap()
    ones4_d = nc.dram_tensor("ones4", [128, 32], F16, kind="ExternalInput").ap()
    small_d = nc.dram_tensor("small", [128, 8], F32, kind="ExternalInput").ap()
    y_d = nc.dram_tensor("y", [32, 1024], F32, kind="ExternalOutput").ap()

    with tile.TileContext(nc) as tc, ExitStack() as ctx, \
            nc.allow_low_precision(reason="fp16 barrier partials validated"):
        cw = ctx.enter_context(tc.tile_pool(name="cw", bufs=1))
        xin = ctx.enter_context(tc.tile_pool(name="xin", bufs=3))
        s8p = ctx.enter_context(tc.tile_pool(name="s8p", bufs=2))
        rgp = ctx.enter_context(tc.tile_pool(name="rgp", bufs=2))
        pep = ctx.enter_context(tc.tile_pool(name="pep", bufs=2))
        bw = ctx.enter_context(tc.tile_pool(name="bw", bufs=1))
        p1 = ctx.enter_context(tc.tile_pool(name="p1", bufs=3, space="PSUM"))
        p2 = ctx.enter_context(tc.tile_pool(name="p2", bufs=1, space="PSUM"))
        p3 = ctx.enter_context(tc.tile_pool(name="p3", bufs=1, space="PSUM"))

        warmt = cw.tile([128, 512], F16)
        nc.gpsimd.memset(warmt[:], 0.0)
        wpsum = p2.tile([128, 512], F32, tag="prh", name="warmps")
        for wi in range(7):
            nc.tensor.matmul(wpsum[:], lhsT=warmt[:, 0:128], rhs=warmt[:],
                             start=True, stop=True)
        w1p = cw.tile([128, 8 * 128], F16); nc.sync.dma_start(w1p[:], w1p_d)
        small = cw.tile([128, 8], F32); nc.sync.dma_start(small[:], small_d)
        ow1p = cw.tile([128, 4 * 128], F16); nc.sync.dma_start(ow1p[:], ow1p_d)
        l2w = cw.tile([128, 256], F8)
        pw = cw.tile([66, 64], F16)
        pw2 = cw.tile([65, 2], F16)
        ones4 = cw.tile([128, 32], F16)
        xb = cw.tile([128, 2 * BARC], F32)

        # barrier working tiles (fp32 until the neighbor reduce)
        sqx = bw.tile([128, BARC], F32)
        sqy = bw.tile([128, BARC], F32)
        n2 = bw.tile([128, BARC], F32)
        sr = bw.tile([128, BARC], F32)
        dd = bw.tile([128, BARC], F32)
        rr = bw.tile([128, BARC], F32)
        rpx = bw.tile([128, BARC], F16)
        rpy = bw.tile([128, BARC], F16)
        barS = bw.tile([32, 1024], F32)
        xbx = xb[:, 0:BARC]
        xby = xb[:, BARC:2 * BARC]

        def bar_step(i):
            if i == 0:
                nc.gpsimd.tensor_tensor(sqx[:], xbx, xbx, op=ALU.mult)
            elif i == 1:
                nc.gpsimd.tensor_tensor(sqy[:], xby, xby, op=ALU.mult)
            elif i == 2:
                nc.gpsimd.tensor_tensor(n2[:], sqx[:], sqy[:], op=ALU.add)
            elif i == 3:
                nc.scalar.activation(sr[:], n2[:], AF.Sqrt)
            elif i == 4:
                nc.gpsimd.tensor_scalar(dd[:], sr[:], 1.0 / B_GAMMA,
                                         -DS / B_GAMMA, op0=ALU.mult, op1=ALU.add)
            elif i == 5:
                nc.vector.reciprocal_approx_fast(out=rr[:], in_=dd[:])
            elif i == 6:
                nc.gpsimd.tensor_tensor(rpx[:], xbx, rr[:], op=ALU.mult)
            elif i == 7:
                nc.gpsimd.tensor_tensor(rpy[:], xby, rr[:], op=ALU.mult)
            elif i == 8:
                bx = p3.tile([32, 400], F32, tag="pb", name="barxp")
                for f in range(4):
                    nc.tensor.matmul(bx[:], lhsT=ones4[:],
                                     rhs=rpx[:, 400 * f:400 * f + 400],
                                     start=(f == 0), stop=(f == 3))
                nc.vector.tensor_copy(barS[:, 0:400], bx[:])
            elif i == 9:
                by = p3.tile([32, 400], F32, tag="pb", name="baryp")
                for f in range(4):
                    nc.tensor.matmul(by[:], lhsT=ones4[:],
                                     rhs=rpy[:, 400 * f:400 * f + 400],
                                     start=(f == 0), stop=(f == 3))
                nc.vector.tensor_copy(barS[:, 512:912], by[:])

        peacc = [None, None]

        def chunk_tail(c):
            et = pep.tile([32, 512], F32, tag="et")
            nc.vector.transpose(et[:], peacc[c][:])
            e1 = pep.tile([32, 512], F32, tag="e1")
            nc.scalar.activation(e1[:], et[:], AF.Tanh)
            act = pep.tile([32, 512], F32, tag="act")
            actv = act[:].rearrange("p (b m u) -> p b m u", m=16, u=2)
            e1v = e1[:].rearrange("p (b m u) -> p b m u", m=16, u=2)
            bxv = barS[:, 256 * c:256 * c + 256].rearrange(
                "p (m b o) -> p b m o", b=16, o=1)
            byv = barS[:, 512 + 256 * c:512 + 256 * c + 256].rearrange(
                "p (m b o) -> p b m o", b=16, o=1)
            nc.vector.tensor_add(actv[:, :, :, 0:1], e1v[:, :, :, 0:1], bxv)
            nc.vector.tensor_add(actv[:, :, :, 1:2], e1v[:, :, :, 1:2], byv)
            yt = pep.tile([32, 512], F32, tag="yt")
            nc.scalar.activation(yt[:], act[:], AF.Tanh)
            y2 = pep.tile([32, 512], F32, tag="y2")
            nc.vector.tensor_scalar_mul(y2[:], yt[:], 2.0)
            nc.sync.dma_start(y_d[:, 512 * c:512 * c + 512], y2[:])

        # 3-stage software pipeline: iteration i runs layer-1(i) interleaved
        # with the fused layer-2 DR matmuls of group i-1 and the psi head of
        # group i-2, so TensorE never waits on a just-issued evac.
        st1 = None   # group i-1 state
        st2 = None   # group i-2 state
        st3 = None   # group i-3 state
        for i in range(G512 + 3):
            cur = None
            if i == 0:
                nc.sync.dma_start(l2w[:], l2w_d)
                nc.sync.dma_start(pw[:], pw_d)
                nc.sync.dma_start(pw2[:], pw2_d)
                nc.sync.dma_start(ones4[:], ones4_d)
            if i == 2:
                nc.sync.dma_start(xb[:], xb_d)
            if i < G512:
                cs = i * 512
                xt = xin.tile([128, 512], F16, tag="xt", name=f"xt{i}")
                nc.sync.dma_start(xt[:], xt_d[:, cs:cs + 512])
                rg = rgp.tile([66, 512], F16, tag="rg", bufs=3, name=f"rg{i}")
                nc.sync.dma_start(rg[64:66, :], gg_d[:, cs:cs + 512])
                parents = [s8p.tile([128, 1024], F8, tag=f"s{j}",
                                    name=f"s8_{i}_{j}") for j in range(6)]
                cur = dict(i=i, xt=xt, rg=rg, parents=parents)

            if st2 is not None:
                g2 = st2["i"]
                pb = p3.tile([128, 512], F32, tag="pb", name=f"pb{g2}")
                st2["pb"] = pb
                nc.tensor.matmul(pb[0:64, :], lhsT=pw[:], rhs=st2["rg"][:],
                                 start=True, stop=True)

            if st1 is not None:
                prh = p2.tile([64, 512], F32, tag="prh", name=f"prh{st1['i']}")
                st1["prh"] = prh

            for j in range(6):
                if cur is not None:
                    ps = p1.tile([128, 1024], F32, tag="ps", name=f"ps_{i}_{j}")
                    for ko in range(2):
                        idx = 2 * j + ko
                        if idx < 8:
                            lhsT = w1p[:, idx * 128:idx * 128 + 128]
                        else:
                            lhsT = ow1p[:, (idx - 8) * 128:(idx - 8) * 128 + 128]
                        nc.tensor.matmul(ps[:, ko * 512:ko * 512 + 512],
                                         lhsT=lhsT, rhs=cur["xt"][:],
                                         start=True, stop=True)
                    bcol = small[:, 0:1] if j < 4 else small[:, 1:2]
                    dst = cur["parents"][j][:]
                    if j % 2 == 0:
                        nc.scalar.activation(dst, ps[:], AF.Relu, bias=bcol)
                    else:
                        nc.vector.tensor_scalar(dst, ps[:], bcol, 0.0,
                                                op0=ALU.add, op1=ALU.max)
                if st1 is not None:
                    rhs = st1["parents"][j][:].rearrange("p (ko n) -> p ko n", ko=2)
                    wsel = l2w[:, 0:128] if j < 4 else l2w[:, 128:256]
                    lhsT = wsel.rearrange("p (ko q) -> p ko q", ko=2)
                    nc.tensor.matmul(st1["prh"][:], lhsT=lhsT, rhs=rhs,
                                     start=(j == 0), stop=(j == 5),
                                     perf_mode=PM.DoubleRow)

            if st1 is not None:
                nc.scalar.activation(st1["rg"][0:64, :], st1["prh"][:], AF.Relu,
                                     bias=small[0:64, 2:3], scale=1.0 / (S1 * S2))
            if st2 is not None:
                phh = rgp.tile([65, 512], F16, tag="phh", name=f"phh{g2}")
                nc.gpsimd.memset(phh[64:65, :], 1.0)
                nc.scalar.activation(phh[0:64, :], st2["pb"][0:64, :], AF.Relu,
                                     bias=small[0:64, 3:4])
                nc.tensor.matmul(st2["pb"][64:66, :], lhsT=pw2[:], rhs=phh[:],
                                 start=True, stop=True, tile_position=(0, 64))
                m2, c2 = g2 % 16, g2 // 16
                if m2 == 0:
                    peacc[c2] = pep.tile([32, 512], F32, tag="pe",
                                         name=f"peacc{c2}")
                petmp = rgp.tile([2, 512], F32, tag="petmp", name=f"petmp{g2}")
                nc.vector.tensor_copy(petmp[:], st2["pb"][64:66, :])
                nc.sync.dma_start(peacc[c2][2 * m2:2 * m2 + 2, :], petmp[:])

            if i < 10:
                bar_step(i)
            if st2 is not None and st2["i"] == 15:
                chunk_tail(0)
            st3, st2, st1 = st2, st1, cur
        chunk_tail(1)
    return nc


_CACHED = {}


def kernel(**inputs):
    x = np.asarray(inputs["x"], np.float32)
    wk = _pack_weights(**{k: np.asarray(v, np.float32) for k, v in inputs.items()
                          if k != "x"})
    in_maps = []
    for core in range(NCORE):
        xp = np.zeros((AP_, D_OBS), np.float32)
        xp[:AC] = x[core * AC:(core + 1) * AC]
        xt = np.zeros((128, AP_), np.float16)
        xt[0:D_OBS] = xp.T.astype(np.float16)
        gg = np.ascontiguousarray(xp[:, 0:2].T.astype(np.float16))
        p = -xp[:, 5:69].reshape(AP_, 16, 4)[:, :, 0:2]       # [A, 16, 2]
        # [gi, bj, a32, nhi, nlo] -> partition 32*nlo + a32, col (gi*16+bj)*4+nhi
        pr = p.reshape(G512, 16, 32, 4, 4, 2)
        xbh = pr.transpose(5, 4, 2, 3, 0, 1).reshape(2, 128, BARC)
        xb = np.ascontiguousarray(
            np.concatenate([xbh[0], xbh[1]], axis=1).astype(np.float32))
        m = dict(wk)
        m["xt"] = np.ascontiguousarray(xt)
        m["gg"] = gg
        m["xb"] = xb
        in_maps.append(m)

    if "nc" not in _CACHED:
        nc = bacc.Bacc("TRN2", target_bir_lowering=False, debug=False,
                       num_devices=NCORE)
        _build(nc)
        nc.compile()
        _CACHED["nc"] = nc
    nc = _CACHED["nc"]
    res = run_bass_kernel_spmd(nc, in_maps, core_ids=list(range(NCORE)))
    _CACHED["last_res"] = res
    out = np.empty((B, ADIM), np.float32)
    for core in range(NCORE):
        Y = res.results[core]["y"]                            # [32, 1024]
        Y5 = Y.reshape(32, 2, 16, 16, 2).transpose(1, 3, 2, 0, 4)
        Y5 = Y5.reshape(32, 512, 2)[:G512].reshape(AP_, 2)
        out[core * AC:(core + 1) * AC] = Y5[:AC]
    return out


if __name__ == "__main__":
    import reference
    ins = {k: np.asarray(v) for k, v in reference.setup_inputs().items()}
    got = kernel(**ins)
    exp = np.asarray(reference.reference(**ins))
    err = np.abs(got - exp).max()
    rel = err / np.abs(exp).max()
    print(f"absmax {err:.4e} rel {rel:.4e}")


# revision 27
# speedup vs baseline: 1.2079x; 1.0102x over previous
"""Barrier_Net TRN2 kernel: 8-core data-parallel Bass/Tile implementation.

Per-core structure (12800 padded agents = 25 groups x 512 columns), built as
a 3-stage software pipeline so TensorE never waits on a just-issued evac:
iteration i runs layer-1(i), the fused layer-2 of group i-1 (interleaved in
the tensor queue), and the psi head of group i-2.

  - Layer-1: 12 fp16 matmuls/group (weights pre-scaled x32); pairs share a
    2-bank [128,1024] PSUM tile so each relu evacuation is one instruction
    (fp8e4 out, values 32*relu), alternating ScalarE/VectorE.
  - Layer-2 + rho1 fused by linearity (lhsT = (w2 @ rho_w1) * 8 in fp8):
    6 DoubleRow fp8 matmuls/group (virtual contract 256) accumulate the rho
    hidden pre-act (x256) into one PSUM tile; the 1/256 folds into the
    rho-relu activation scale.
  - rho2 + psi1 fused (lhsT rows 0:64 = rho_w2 @ psi_w1[:2]); g is DMA'd
    into rows 64:66 of the same rhs tile -> a single contract-66 matmul.
    psi_b2 rides a const-1 row appended to phh (lhsT row 64 of pw2).
  - empty-head e is staged per group into a [32,512] accumulator (16 groups
    per chunk), transposed once per chunk, tanh'd in batch; only one
    act-table switch in the whole kernel (sqrt-set -> exp-set at chunk 0).
  - barrier computed fully in fp32 (accuracy requires it: fp16 anywhere in
    p/n2/sqrt costs 3e-2 rel err) in a [128,1600]x2 strip layout; squares,
    h, and r*p run on the otherwise-idle GpSimd; the 16-neighbor sum is
    4 accumulating [32,400] matmuls per component against a block-ones lhsT.
  - ~4us of dummy matmuls at boot warm the PE HAM clock gate (1.2->2.4 GHz)
    while the input DMAs stream.
"""
import sys, os
sys.path.insert(0, "/opt/trn_rl_repo")
import numpy as np
import ml_dtypes
import concourse.bacc as bacc
import concourse.tile as tile
import concourse.mybir as mybir
from concourse.bass_utils import run_bass_kernel_spmd
from contextlib import ExitStack

F32 = mybir.dt.float32
F16 = mybir.dt.float16
F8 = mybir.dt.float8e4
AF = mybir.ActivationFunctionType
ALU = mybir.AluOpType
PM = mybir.MatmulPerfMode

B, NN, NO, SD = 100000, 16, 8, 4
H, PHI_OUT, ADIM = 64, 16, 2
DS, B_GAMMA = 0.2, 0.01
D_OBS = 85
NCORE = 8
AC = B // NCORE            # 12500 agents per core
G512 = 25                  # groups of 512
AP_ = G512 * 512           # padded agents per core = 12800
S1 = 32.0                  # layer-1 weight prescale
S2 = 8.0                   # fused layer-2 weight prescale
BARC = G512 * 16 * 4       # 1600 barrier cols per half

SCALAR_EVACS = frozenset({0, 1, 3, 4, 6, 7, 9, 10})


def _pack_weights(phi_w1, phi_b1, phi_w2, phi_b2, obs_w1, obs_b1, obs_w2, obs_b2,
                  rho_w1, rho_b1, rho_w2, rho_b2, psi_w1, psi_b1, psi_w2, psi_b2):
    # layer-1 block-diagonal lhsT, 2 elements per matmul, prescaled x32
    W1P = np.zeros((8, 128, 128), np.float32)
    for k in range(8):
        for j in range(2):
            n = 2 * k + j
            W1P[k, 5 + 4 * n:5 + 4 * n + 4, 64 * j:64 * j + 64] = phi_w1 * S1
    OW1P = np.zeros((4, 128, 128), np.float32)
    for m in range(4):
        for j in range(2):
            o = 2 * m + j
            OW1P[m, 69 + 2 * o:69 + 2 * o + 2, 64 * j:64 * j + 64] = obs_w1 * S1
    # fused layer-2 + rho1 (DoubleRow lhsT [128, ko=2, 64]), x8
    W2R = np.tile((phi_w2 @ rho_w1) * S2, (2, 1))       # [128, 64]
    OW2R = np.tile((obs_w2 @ rho_w1) * S2, (2, 1))
    l2w = np.zeros((128, 2, 2, 64), np.float32)          # [p, which, ko, j]
    l2w[:, 0, 0] = l2w[:, 0, 1] = W2R
    l2w[:, 1, 0] = l2w[:, 1, 1] = OW2R
    # fused rho2+psi1 lhsT [66, 64]; rh carries a 1/(S1*S2) scale already
    PW = np.zeros((66, 64), np.float32)
    PW[0:64] = rho_w2 @ psi_w1[0:2]
    PW[64:66] = psi_w1[2:4]
    # bias columns
    RB1C = (rho_b1 + (NN * phi_b2 + NO * obs_b2) @ rho_w1).reshape(64, 1)
    PB1C = (psi_b1 + rho_b2 @ psi_w1[0:2]).reshape(64, 1)
    PB2C = psi_b2.reshape(2, 1)
    small = np.zeros((128, 8), np.float32)
    small[:, 0:1] = np.tile(phi_b1, 2).reshape(128, 1) * S1
    small[:, 1:2] = np.tile(obs_b1, 2).reshape(128, 1) * S1
    small[0:64, 2:3] = RB1C
    small[0:64, 3:4] = PB1C
    small[0:2, 4:5] = PB2C
    ones4 = np.zeros((128, 32), np.float32)
    for p in range(128):
        ones4[p, p % 32] = 1.0
    f8 = ml_dtypes.float8_e4m3
    PW2B = np.concatenate([psi_w2, psi_b2.reshape(1, 2)], axis=0)
    return {
        "w1p": W1P.transpose(1, 0, 2).reshape(128, 8 * 128).astype(np.float16),
        "ow1p": OW1P.transpose(1, 0, 2).reshape(128, 4 * 128).astype(np.float16),
        "l2w": l2w.reshape(128, 256).astype(f8),
        "pw": PW.astype(np.float16),
        "pw2": PW2B.astype(np.float16),
        "ones4": ones4.astype(np.float16),
        "small": small.astype(np.float32),
    }


def _build(nc):
    xt_d = nc.dram_tensor("xt", [128, AP_], F16, kind="ExternalInput").ap()
    gg_d = nc.dram_tensor("gg", [2, AP_], F16, kind="ExternalInput").ap()
    xb_d = nc.dram_tensor("xb", [128, 2 * BARC], F32, kind="ExternalInput").ap()
    w1p_d = nc.dram_tensor("w1p", [128, 8 * 128], F16, kind="ExternalInput").ap()
    ow1p_d = nc.dram_tensor("ow1p", [128, 4 * 128], F16, kind="ExternalInput").ap()
    l2w_d = nc.dram_tensor("l2w", [128, 256], F8, kind="ExternalInput").ap()
    pw_d = nc.dram_tensor("pw", [66, 64], F16, kind="ExternalInput").ap()
    pw2_d = nc.dram_tensor("pw2", [65, 2], F16, kind="ExternalInput").ap()
    ones4_d = nc.dram_tensor("ones4", [128, 32], F16, kind="ExternalInput").ap()
    small_d = nc.dram_tensor("small", [128, 8], F32, kind="ExternalInput").ap()
    y_d = nc.dram_tensor("y", [32, 1024], F32, kind="ExternalOutput").ap()

    with tile.TileContext(nc) as tc, ExitStack() as ctx, \
            nc.allow_low_precision(reason="fp16 barrier partials validated"):
        cw = ctx.enter_context(tc.tile_pool(name="cw", bufs=1))
        xin = ctx.enter_context(tc.tile_pool(name="xin", bufs=3))
        s8p = ctx.enter_context(tc.tile_pool(name="s8p", bufs=2))
        rgp = ctx.enter_context(tc.tile_pool(name="rgp", bufs=2))
        pep = ctx.enter_context(tc.tile_pool(name="pep", bufs=2))
        bw = ctx.enter_context(tc.tile_pool(name="bw", bufs=1))
        p1 = ctx.enter_context(tc.tile_pool(name="p1", bufs=3, space="PSUM"))
        p2 = ctx.enter_context(tc.tile_pool(name="p2", bufs=1, space="PSUM"))
        p3 = ctx.enter_context(tc.tile_pool(name="p3", bufs=1, space="PSUM"))

        warmt = cw.tile([128, 512], F16)
        nc.gpsimd.memset(warmt[:], 0.0)
        wpsum = p2.tile([128, 512], F32, tag="prh", name="warmps")
        for wi in range(7):
            nc.tensor.matmul(wpsum[:], lhsT=warmt[:, 0:128], rhs=warmt[:],
                             start=True, stop=True)
        w1p = cw.tile([128, 8 * 128], F16); nc.sync.dma_start(w1p[:], w1p_d)
        small = cw.tile([128, 8], F32); nc.sync.dma_start(small[:], small_d)
        ow1p = cw.tile([128, 4 * 128], F16); nc.sync.dma_start(ow1p[:], ow1p_d)
        l2w = cw.tile([128, 256], F8)
        pw = cw.tile([66, 64], F16)
        pw2 = cw.tile([65, 2], F16)
        ones4 = cw.tile([128, 32], F16)
        xb = cw.tile([128, 2 * BARC], F32)

        # barrier working tiles (fp32 until the neighbor reduce)
        sqx = bw.tile([128, BARC], F32)
        sqy = bw.tile([128, BARC], F32)
        n2 = bw.tile([128, BARC], F32)
        sr = bw.tile([128, BARC], F32)
        dd = bw.tile([128, BARC], F32)
        rr = bw.tile([128, BARC], F32)
        rpx = bw.tile([128, BARC], F16)
        rpy = bw.tile([128, BARC], F16)
        barS = bw.tile([32, 1024], F32)
        xbx = xb[:, 0:BARC]
        xby = xb[:, BARC:2 * BARC]

        def bar_step(i):
            if i == 0:
                nc.gpsimd.tensor_tensor(sqx[:], xbx, xbx, op=ALU.mult)
            elif i == 1:
                nc.gpsimd.tensor_tensor(sqy[:], xby, xby, op=ALU.mult)
            elif i == 2:
                nc.gpsimd.tensor_tensor(n2[:], sqx[:], sqy[:], op=ALU.add)
            elif i == 3:
                nc.scalar.activation(sr[:], n2[:], AF.Sqrt)
            elif i == 4:
                nc.gpsimd.tensor_scalar(dd[:], sr[:], 1.0 / B_GAMMA,
                                         -DS / B_GAMMA, op0=ALU.mult, op1=ALU.add)
            elif i == 5:
                nc.vector.reciprocal_approx_fast(out=rr[:], in_=dd[:])
            elif i == 6:
                nc.gpsimd.tensor_tensor(rpx[:], xbx, rr[:], op=ALU.mult)
            elif i == 7:
                nc.gpsimd.tensor_tensor(rpy[:], xby, rr[:], op=ALU.mult)
            elif i == 8:
                bx = p3.tile([32, 400], F32, tag="pb", name="barxp")
                for f in range(4):
                    nc.tensor.matmul(bx[:], lhsT=ones4[:],
                                     rhs=rpx[:, 400 * f:400 * f + 400],
                                     start=(f == 0), stop=(f == 3))
                nc.vector.tensor_copy(barS[:, 0:400], bx[:])
            elif i == 9:
                by = p3.tile([32, 400], F32, tag="pb", name="baryp")
                for f in range(4):
                    nc.tensor.matmul(by[:], lhsT=ones4[:],
                                     rhs=rpy[:, 400 * f:400 * f + 400],
                                     start=(f == 0), stop=(f == 3))
                nc.vector.tensor_copy(barS[:, 512:912], by[:])

        peacc = [None, None]

        def chunk_tail(c):
            et = pep.tile([32, 512], F32, tag="et")
            nc.vector.transpose(et[:], peacc[c][:])
            e1 = pep.tile([32, 512], F32, tag="e1")
            nc.scalar.activation(e1[:], et[:], AF.Tanh)
            act = pep.tile([32, 512], F32, tag="act")
            actv = act[:].rearrange("p (b m u) -> p b m u", m=16, u=2)
            e1v = e1[:].rearrange("p (b m u) -> p b m u", m=16, u=2)
            bxv = barS[:, 256 * c:256 * c + 256].rearrange(
                "p (m b o) -> p b m o", b=16, o=1)
            byv = barS[:, 512 + 256 * c:512 + 256 * c + 256].rearrange(
                "p (m b o) -> p b m o", b=16, o=1)
            nc.vector.tensor_add(actv[:, :, :, 0:1], e1v[:, :, :, 0:1], bxv)
            nc.vector.tensor_add(actv[:, :, :, 1:2], e1v[:, :, :, 1:2], byv)
            yt = pep.tile([32, 512], F32, tag="yt")
            nc.scalar.activation(yt[:], act[:], AF.Tanh)
            y2 = pep.tile([32, 512], F32, tag="y2")
            nc.vector.tensor_scalar_mul(y2[:], yt[:], 2.0)
            nc.sync.dma_start(y_d[:, 512 * c:512 * c + 512], y2[:])

        # 3-stage software pipeline: iteration i runs layer-1(i) interleaved
        # with the fused layer-2 DR matmuls of group i-1 and the psi head of
        # group i-2, so TensorE never waits on a just-issued evac.
        st1 = None   # group i-1 state
        st2 = None   # group i-2 state
        st3 = None   # group i-3 state
        for i in range(G512 + 3):
            cur = None
            if i == 0:
                nc.sync.dma_start(l2w[:], l2w_d)
                nc.sync.dma_start(pw[:], pw_d)
                nc.sync.dma_start(pw2[:], pw2_d)
                nc.sync.dma_start(ones4[:], ones4_d)
            if i == 2:
                nc.sync.dma_start(xb[:], xb_d)
            if i < G512:
                cs = i * 512
                xt = xin.tile([128, 512], F16, tag="xt", name=f"xt{i}")
                nc.sync.dma_start(xt[:], xt_d[:, cs:cs + 512])
                rg = rgp.tile([66, 512], F16, tag="rg", bufs=3, name=f"rg{i}")
                nc.sync.dma_start(rg[64:66, :], gg_d[:, cs:cs + 512])
                parents = [s8p.tile([128, 1024], F8, tag=f"s{j}",
                                    name=f"s8_{i}_{j}") for j in range(6)]
                cur = dict(i=i, xt=xt, rg=rg, parents=parents)

            if st2 is not None:
                g2 = st2["i"]
                pb = p3.tile([128, 512], F32, tag="pb", name=f"pb{g2}")
                st2["pb"] = pb
                nc.tensor.matmul(pb[0:64, :], lhsT=pw[:], rhs=st2["rg"][:],
                                 start=True, stop=True)

            if st1 is not None:
                prh = p2.tile([64, 512], F32, tag="prh", name=f"prh{st1['i']}")
                st1["prh"] = prh

            for j in range(6):
                if cur is not None:
                    ps = p1.tile([128, 1024], F32, tag="ps", name=f"ps_{i}_{j}")
                    for ko in range(2):
                        idx = 2 * j + ko
                        if idx < 8:
                            lhsT = w1p[:, idx * 128:idx * 128 + 128]
                        else:
                            lhsT = ow1p[:, (idx - 8) * 128:(idx - 8) * 128 + 128]
                        nc.tensor.matmul(ps[:, ko * 512:ko * 512 + 512],
                                         lhsT=lhsT, rhs=cur["xt"][:],
                                         start=True, stop=True)
                    bcol = small[:, 0:1] if j < 4 else small[:, 1:2]
                    dst = cur["parents"][j][:]
                    if j % 2 == 0:
                        nc.scalar.activation(dst, ps[:], AF.Relu, bias=bcol)
                    else:
                        nc.vector.tensor_scalar(dst, ps[:], bcol, 0.0,
                                                op0=ALU.add, op1=ALU.max)
                if st1 is not None:
                    rhs = st1["parents"][j][:].rearrange("p (ko n) -> p ko n", ko=2)
                    wsel = l2w[:, 0:128] if j < 4 else l2w[:, 128:256]
                    lhsT = wsel.rearrange("p (ko q) -> p ko q", ko=2)
                    nc.tensor.matmul(st1["prh"][:], lhsT=lhsT, rhs=rhs,
                                     start=(j == 0), stop=(j == 5),
                                     perf_mode=PM.DoubleRow)

            if st1 is not None:
                nc.scalar.activation(st1["rg"][0:64, :], st1["prh"][:], AF.Relu,
                                     bias=small[0:64, 2:3], scale=1.0 / (S1 * S2))
            if st2 is not None:
                phh = rgp.tile([65, 512], F16, tag="phh", name=f"phh{g2}")
                nc.gpsimd.memset(phh[64:65, :], 1.0)
                nc.scalar.activation(phh[0:64, :], st2["pb"][0:64, :], AF.Relu,
                                     bias=small[0:64, 3:4])
                nc.tensor.matmul(st2["pb"][64:66, :], lhsT=pw2[:], rhs=phh[:],
                                 start=True, stop=True, tile_position=(0, 64))
                m2, c2 = g2 % 16, g2 // 16
                if m2 == 0:
                    peacc[c2] = pep.tile([32, 512], F32, tag="pe",
                                         name=f"peacc{c2}")
                petmp = rgp.tile([2, 512], F32, tag="petmp", name=f"petmp{g2}")
                nc.vector.tensor_copy(petmp[:], st2["pb"][64:66, :])
                nc.sync.dma_start(peacc[c2][2 * m2:2 * m2 + 2, :], petmp[:])

            if i < 10:
                bar_step(i)
            if st2 is not None and st2["i"] == 15:
                chunk_tail(0)
            st3, st2, st1 = st2, st1, cur
        chunk_tail(1)
    return nc


_CACHED = {}


def kernel(**inputs):
    x = np.asarray(inputs["x"], np.float32)
    wk = _pack_weights(**{k: np.asarray(v, np.float32) for k, v in inputs.items()
                          if k != "x"})
    in_maps = []
    for core in range(NCORE):
        xp = np.zeros((AP_, D_OBS), np.float32)
        xp[:AC] = x[core * AC:(core + 1) * AC]
        xt = np.zeros((128, AP_), np.float16)
        xt[0:D_OBS] = xp.T.astype(np.float16)
        gg = np.ascontiguousarray(xp[:, 0:2].T.astype(np.float16))
        p = -xp[:, 5:69].reshape(AP_, 16, 4)[:, :, 0:2]       # [A, 16, 2]
        # [gi, bj, a32, nhi, nlo] -> partition 32*nlo + a32, col (gi*16+bj)*4+nhi
        pr = p.reshape(G512, 16, 32, 4, 4, 2)
        xbh = pr.transpose(5, 4, 2, 3, 0, 1).reshape(2, 128, BARC)
        xb = np.ascontiguousarray(
            np.concatenate([xbh[0], xbh[1]], axis=1).astype(np.float32))
        m = dict(wk)
        m["xt"] = np.ascontiguousarray(xt)
        m["gg"] = gg
        m["xb"] = xb
        in_maps.append(m)

    if "nc" not in _CACHED:
        nc = bacc.Bacc("TRN2", target_bir_lowering=False, debug=False,
                       num_devices=NCORE)
        _build(nc)
        nc.compile()
        _CACHED["nc"] = nc
    nc = _CACHED["nc"]
    res = run_bass_kernel_spmd(nc, in_maps, core_ids=list(range(NCORE)))
    _CACHED["last_res"] = res
    out = np.empty((B, ADIM), np.float32)
    for core in range(NCORE):
        Y = res.results[core]["y"]                            # [32, 1024]
        Y5 = Y.reshape(32, 2, 16, 16, 2).transpose(1, 3, 2, 0, 4)
        Y5 = Y5.reshape(32, 512, 2)[:G512].reshape(AP_, 2)
        out[core * AC:(core + 1) * AC] = Y5[:AC]
    return out


if __name__ == "__main__":
    import reference
    ins = {k: np.asarray(v) for k, v in reference.setup_inputs().items()}
    got = kernel(**ins)
    exp = np.asarray(reference.reference(**ins))
    err = np.abs(got - exp).max()
    rel = err / np.abs(exp).max()
    print(f"absmax {err:.4e} rel {rel:.4e}")
